# revision 31
# baseline (speedup 1.0000x reference)
"""Cross-attention Trainium2 kernel (self-contained).

Reference computation (B=4, N=M=2048, DIM=1024, H=16, Dh=64):
    q = x @ Wq.T ; k = ctx @ Wk.T ; v = ctx @ Wv.T       (per-head split)
    out = softmax(q k^T / sqrt(Dh)) v                     (per b, h)
    final = out @ Wo.T + bo

Sharding over 8 NeuronCores: core c -> (batch b = c//2, head-group g = c%2).
Each core handles 8 heads (512 of the 1024 inner dims) of one batch and
produces a partial (2048, 1024) output-projection contribution; the host sums
the two partials per batch and adds the bias.

On-chip dataflow keeps every matmul contraction on the partition axis:
    Q^T = (Wq_g^T as lhsT stacks) with x^T as moving operand -> (d, n)
    K^T likewise -> (d, m);  V -> (m, d) with a ones-column per head so the
    attn@V matmul also emits softmax denominators.
    scores^T (m, n) per head via K=64 matmuls, two heads packed in the
    128-row PE array; exp on ScalarE with fused 1/sqrt(Dh) scale (max |logit|
    = 3.8, so no max-subtraction needed); denominator reciprocal broadcast via
    a rank-1 PE matmul.
"""

import numpy as np
import ml_dtypes
from contextlib import ExitStack

import concourse.bass as bass
import concourse.bacc as bacc
import concourse.tile as tile
from concourse import mybir
from concourse import bass_utils

F32 = mybir.dt.float32
BF16 = mybir.dt.bfloat16

B, N, M, DIM = 4, 2048, 2048, 1024
H, DH = 16, 64
NCORES = 8
HG = DIM // 2          # head dims per core (8 heads * 64)
SCALE = DH ** -0.5

_CACHE = {}


def _build_program():
    nc = bacc.Bacc(
        "TRN2",
        target_bir_lowering=False,
        debug=False,
        enable_asserts=False,
        num_devices=NCORES,
    )
    xT = nc.dram_tensor("xT", (DIM, N), BF16, kind="ExternalInput").ap()
    ctxT = nc.dram_tensor("ctxT", (DIM, M), BF16, kind="ExternalInput").ap()
    wqT = nc.dram_tensor("wqT", (DIM, HG), BF16, kind="ExternalInput").ap()
    wkT = nc.dram_tensor("wkT", (DIM, HG), BF16, kind="ExternalInput").ap()
    wvT = nc.dram_tensor("wvT", (DIM, HG), BF16, kind="ExternalInput").ap()
    woT = nc.dram_tensor("woT", (HG, DIM), BF16, kind="ExternalInput").ap()
    out = nc.dram_tensor("out", (N, DIM), F32, kind="ExternalOutput").ap()

    with tile.TileContext(nc) as tc:
        _kernel_body(tc, xT, ctxT, wqT, wkT, wvT, woT, out)
    nc.compile()
    return nc


def _kernel_body(tc, xT, ctxT, wqT, wkT, wvT, woT, out):
    nc = tc.nc
    EXP = mybir.ActivationFunctionType.Exp
    NT = N // 512       # q-row tiles of 512
    MT = M // 128       # context-row tiles of 128
    CT = DIM // 128     # contraction tiles for projections
    DT = HG // 128      # head-dim tiles per core (= head pairs)

    with ExitStack() as ctx:
        sb = ctx.enter_context(tc.tile_pool(name="sb", bufs=1))

        xT_sb = sb.tile([128, CT, N], BF16, tag="xT")
        ctxT_sb = sb.tile([128, CT, M], BF16, tag="ctxT")
        wq_sb = sb.tile([128, CT, HG], BF16, tag="wq")
        wk_sb = sb.tile([128, CT, HG], BF16, tag="wk")
        wv_sb = sb.tile([128, CT, HG], BF16, tag="wv")
        wo_sb = sb.tile([128, DT, DIM], BF16, tag="wo")
        qT_sb = sb.tile([128, DT, N], BF16, tag="qT")
        kT_sb = sb.tile([128, DT, M], BF16, tag="kT")
        v_sb = sb.tile([128, MT, 8 * 65], BF16, tag="v")
        on_sb = sb.tile([128, DT, N], BF16, tag="on")

        # ---- loads (host passes bf16) ----
        for c in range(CT):
            nc.sync.dma_start(out=wq_sb[:, c, :], in_=wqT[c * 128:(c + 1) * 128, :])
        for c in range(CT):
            nc.sync.dma_start(out=xT_sb[:, c, :], in_=xT[c * 128:(c + 1) * 128, :])
        for c in range(CT):
            nc.sync.dma_start(out=wk_sb[:, c, :], in_=wkT[c * 128:(c + 1) * 128, :])
        for c in range(CT):
            nc.sync.dma_start(out=ctxT_sb[:, c, :], in_=ctxT[c * 128:(c + 1) * 128, :])
        for c in range(CT):
            nc.sync.dma_start(out=wv_sb[:, c, :], in_=wvT[c * 128:(c + 1) * 128, :])
        for t in range(DT):
            nc.sync.dma_start(out=wo_sb[:, t, :], in_=woT[t * 128:(t + 1) * 128, :])

        v_r = v_sb.rearrange("p m (h x) -> p m h x", x=65)
        for h in range(8):
            nc.vector.memset(v_r[:, :, h, 64:65], 1.0)

        # ---- compute: projections interleaved into attention ----
        # Pair 0's Q/K/V projections run up front; while attention for pair
        # pr streams (ScalarE-bound), the PE's idle slots are filled with
        # pair pr+1's projections, and during the last pair with the output
        # projection of already-normalized row blocks.
        psp = ctx.enter_context(tc.tile_pool(name="psp", bufs=1, space="PSUM"))
        pss = ctx.enter_context(tc.tile_pool(name="pss", bufs=2, space="PSUM"))
        pso = ctx.enter_context(tc.tile_pool(name="pso", bufs=3, space="PSUM"))
        sba = ctx.enter_context(tc.tile_pool(name="sba", bufs=6))
        sbn = ctx.enter_context(tc.tile_pool(name="sbn", bufs=4))
        sbo = ctx.enter_context(tc.tile_pool(name="sbo", bufs=3))

        def q_group(pr, jn):
            ps = psp.tile([128, 512], F32, tag="proj", name="qg")
            for c in range(CT):
                nc.tensor.matmul(
                    ps,
                    wq_sb[:, c, pr * 128:(pr + 1) * 128],
                    xT_sb[:, c, jn * 512:(jn + 1) * 512],
                    start=(c == 0), stop=(c == CT - 1),
                )
                if c % 3 == 2:
                    yield
            nc.vector.tensor_copy(qT_sb[:, pr, jn * 512:(jn + 1) * 512], ps)

        def k_group(pr, jm):
            ps = psp.tile([128, 512], F32, tag="proj", name="kg")
            for c in range(CT):
                nc.tensor.matmul(
                    ps,
                    wk_sb[:, c, pr * 128:(pr + 1) * 128],
                    ctxT_sb[:, c, jm * 512:(jm + 1) * 512],
                    start=(c == 0), stop=(c == CT - 1),
                )
                if c % 3 == 2:
                    yield
            nc.vector.tensor_copy(kT_sb[:, pr, jm * 512:(jm + 1) * 512], ps)

        def v_group(cp, i):
            # one couple = two head pairs = 4 heads (256 projection dims)
            ps = psp.tile([128, 256], F32, tag="proj", name="vg")
            for c in range(CT):
                nc.tensor.matmul(
                    ps,
                    ctxT_sb[:, c, i * 128:(i + 1) * 128],
                    wv_sb[:, c, cp * 256:(cp + 1) * 256],
                    start=(c == 0), stop=(c == CT - 1),
                )
                if c % 3 == 2:
                    yield
            nc.vector.tensor_copy(
                v_r[:, i, 4 * cp:4 * cp + 4, 0:64],
                ps.rearrange("p (h d) -> p h d", h=4),
            )

        def final_group(n128, e):
            ps = psp.tile([128, 512], F32, tag="proj", name="fg")
            for t in range(DT):
                nc.tensor.matmul(
                    ps,
                    on_sb[:, t, n128 * 128:(n128 + 1) * 128],
                    wo_sb[:, t, e * 512:(e + 1) * 512],
                    start=(t == 0), stop=(t == DT - 1),
                )
                if t == 1:
                    yield
            of = sbo.tile([128, 512], F32, tag="of", name="of")
            nc.vector.tensor_copy(of, ps)
            nc.sync.dma_start(
                out=out[n128 * 128:(n128 + 1) * 128, e * 512:(e + 1) * 512],
                in_=of,
            )

        def proj_pair_gens(pr, with_v):
            gens = ([q_group(pr, jn) for jn in range(NT)]
                    + [k_group(pr, jm) for jm in range(M // 512)])
            if with_v:
                # V for the NEXT couple (pairs 2,3), queued while pair 0 runs
                gens += [v_group(1, i) for i in range(MT)]
            return gens

        class Pacer:
            def __init__(self):
                self.queue = []
                self.cur = None

            def step_group(self):
                # emit one whole projection group (all its chunks)
                if self.cur is not None:
                    for _ in self.cur:
                        pass
                    self.cur = None
                elif self.queue:
                    for _ in self.queue.pop(0):
                        pass

            def step(self, n=1):
                for _ in range(n):
                    while True:
                        if self.cur is None:
                            if not self.queue:
                                return
                            self.cur = self.queue.pop(0)
                        try:
                            next(self.cur)
                            break
                        except StopIteration:
                            self.cur = None

            def drain(self):
                while self.cur is not None or self.queue:
                    self.step()

        def scores(pr, j, i):
            s = pss.tile([128, 1024], F32, tag="sc", name="sc")
            for half in range(2):
                lo, hi = half * 64, half * 64 + 64
                nc.tensor.matmul(
                    s[:, half * 512:(half + 1) * 512],
                    kT_sb[lo:hi, pr, i * 128:(i + 1) * 128],
                    qT_sb[lo:hi, pr, j * 512:(j + 1) * 512],
                    start=True, stop=True,
                )
            return s

        def exp_av(oo, pr, i, s):
            a = sba.tile([128, 1024], BF16, tag="attn", name="attn")
            nc.scalar.activation(a, s, EXP, scale=SCALE)
            for half in range(2):
                nc.tensor.matmul(
                    oo[half],
                    v_r[:, i, 2 * pr + half, :],
                    a[:, half * 512:(half + 1) * 512],
                    start=(i == 0), stop=(i == MT - 1),
                )

        def normalize(oo, pr, j):
            # rows 0..63 of oacc are sum(attn*v); row 64 is sum(attn).
            # Whole chain runs on DVE + GpSimd, off the PE/ACT critical path;
            # both reciprocals issue before the muls so the GpSimd broadcasts
            # overlap the second half's DVE work.
            bcs = []
            for half in range(2):
                den = sbn.tile([1, 512], F32, tag="den", name="den")
                nc.vector.tensor_copy(den, oo[half][64:65, :])
                rec32 = sbn.tile([1, 512], F32, tag="rec32", name="rec32")
                nc.vector.reciprocal_approx_fast(out=rec32, in_=den)
                bc = sbn.tile([64, 512], F32, tag="bc", name="bc")
                nc.gpsimd.partition_broadcast(bc, rec32)
                bcs.append(bc)
            for half in range(2):
                nc.vector.tensor_mul(
                    on_sb[half * 64:half * 64 + 64, pr, j * 512:(j + 1) * 512],
                    oo[half][0:64, :], bcs[half],
                )

        # pair 0 projections + V for couple 0 (heads 0-3) up front
        for g in proj_pair_gens(0, False) + [v_group(0, i) for i in range(MT)]:
            for _ in g:
                pass

        pacer = Pacer()
        for pr in range(DT):
            if pr + 1 < DT:
                pacer.queue.extend(proj_pair_gens(pr + 1, with_v=(pr == 0)))
            for j in range(NT):
                if pr == DT - 1 and j >= 1:
                    jj = j - 1  # tile (pr, jj) was normalized at its end
                    pacer.queue.extend(
                        final_group(n128, e)
                        for n128 in range(jj * 4, jj * 4 + 4)
                        for e in range(2))
                oo = [pso.tile([65, 512], F32, tag="oacc", name=f"oacc{h}")
                      for h in range(2)]
                fifo = [scores(pr, j, 0), scores(pr, j, 1)]
                for i in range(MT):
                    if i + 2 < MT:
                        fifo.append(scores(pr, j, i + 2))
                    exp_av(oo, pr, i, fifo.pop(0))
                    if i % 2 == 1:
                        pacer.step_group()
                normalize(oo, pr, j)
            pacer.drain()
        # last row block's output projection
        for n128 in range(12, 16):
            for e in range(2):
                for _ in final_group(n128, e):
                    pass


def kernel(x, context, Wq, Wk, Wv, Wo, bo):
    x = np.asarray(x, dtype=np.float32)
    context = np.asarray(context, dtype=np.float32)
    Wq = np.asarray(Wq, dtype=np.float32)
    Wk = np.asarray(Wk, dtype=np.float32)
    Wv = np.asarray(Wv, dtype=np.float32)
    Wo = np.asarray(Wo, dtype=np.float32)
    bo = np.asarray(bo, dtype=np.float32)

    if "nc" not in _CACHE:
        _CACHE["nc"] = _build_program()
    nc = _CACHE["nc"]

    in_maps = _make_in_maps(x, context, Wq, Wk, Wv, Wo)
    res = bass_utils.run_bass_kernel_spmd(nc, in_maps, core_ids=list(range(NCORES)))

    final = np.empty((B, N, DIM), dtype=np.float32)
    for b in range(B):
        final[b] = res.results[2 * b]["out"] + res.results[2 * b + 1]["out"] + bo
    return final


def _make_in_maps(x, context, Wq, Wk, Wv, Wo):
    bf = ml_dtypes.bfloat16
    xT = [np.ascontiguousarray(x[b].T).astype(bf) for b in range(B)]
    ctxT = [np.ascontiguousarray(context[b].T).astype(bf) for b in range(B)]
    wT = {}
    for g in range(2):
        sl = slice(g * HG, (g + 1) * HG)
        wT[g] = {
            "wqT": np.ascontiguousarray(Wq[sl, :].T).astype(bf),
            "wkT": np.ascontiguousarray(Wk[sl, :].T).astype(bf),
            "wvT": np.ascontiguousarray(Wv[sl, :].T).astype(bf),
            "woT": np.ascontiguousarray(Wo[:, sl].T).astype(bf),
        }
    in_maps = []
    for c in range(NCORES):
        b, g = c // 2, c % 2
        m = {"xT": xT[b], "ctxT": ctxT[b]}
        m.update(wT[g])
        in_maps.append(m)
    return in_maps


def timed_run(inp, trace_dir=None):
    """Run with NTFF tracing; returns HW exec time in ns (or None)."""
    if "nc" not in _CACHE:
        _CACHE["nc"] = _build_program()
    nc = _CACHE["nc"]
    in_maps = _make_in_maps(
        np.asarray(inp["x"], np.float32), np.asarray(inp["context"], np.float32),
        np.asarray(inp["Wq"], np.float32), np.asarray(inp["Wk"], np.float32),
        np.asarray(inp["Wv"], np.float32), np.asarray(inp["Wo"], np.float32))
    res = bass_utils.run_bass_kernel_spmd(
        nc, in_maps, core_ids=list(range(NCORES)), trace=True, tmpdir=trace_dir)
    return res.exec_time_ns


# revision 32
# speedup vs baseline: 1.1784x; 1.1784x over previous
"""Cross-attention Trainium2 kernel (self-contained).

Reference computation (B=4, N=M=2048, DIM=1024, H=16, Dh=64):
    q = x @ Wq.T ; k = ctx @ Wk.T ; v = ctx @ Wv.T       (per-head split)
    out = softmax(q k^T / sqrt(Dh)) v                     (per b, h)
    final = out @ Wo.T + bo

Sharding over 8 NeuronCores: core c -> (batch b = c//2, head-group g = c%2).
Each core handles 8 heads (512 of the 1024 inner dims) of one batch and
produces a partial (2048, 1024) output-projection contribution; the host sums
the two partials per batch and adds the bias.

On-chip dataflow keeps every matmul contraction on the partition axis:
    Q^T = (Wq_g^T as lhsT stacks) with x^T as moving operand -> (d, n)
    K^T likewise -> (d, m);  V -> (m, d) with a ones-column per head so the
    attn@V matmul also emits softmax denominators.
    scores^T (m, n) per head via K=64 matmuls, two heads packed in the
    128-row PE array; exp on ScalarE with fused 1/sqrt(Dh) scale (max |logit|
    = 3.8, so no max-subtraction needed); denominator reciprocal broadcast via
    a rank-1 PE matmul.
"""

import numpy as np
import ml_dtypes
from contextlib import ExitStack

import concourse.bass as bass
import concourse.bacc as bacc
import concourse.tile as tile
from concourse import mybir
from concourse import bass_utils

F32 = mybir.dt.float32
BF16 = mybir.dt.bfloat16

B, N, M, DIM = 4, 2048, 2048, 1024
H, DH = 16, 64
NCORES = 8
HG = DIM // 2          # head dims per core (8 heads * 64)
SCALE = DH ** -0.5

_CACHE = {}


def _build_program():
    nc = bacc.Bacc(
        "TRN2",
        target_bir_lowering=False,
        debug=False,
        enable_asserts=False,
        num_devices=NCORES,
    )
    xT = nc.dram_tensor("xT", (DIM, N), BF16, kind="ExternalInput").ap()
    ctxT = nc.dram_tensor("ctxT", (DIM, M), BF16, kind="ExternalInput").ap()
    wqT = nc.dram_tensor("wqT", (DIM, HG), BF16, kind="ExternalInput").ap()
    wkT = nc.dram_tensor("wkT", (DIM, HG), BF16, kind="ExternalInput").ap()
    wvT = nc.dram_tensor("wvT", (DIM, HG), BF16, kind="ExternalInput").ap()
    woT = nc.dram_tensor("woT", (HG, DIM), BF16, kind="ExternalInput").ap()
    out = nc.dram_tensor("out", (N, DIM), F32, kind="ExternalOutput").ap()

    with tile.TileContext(nc) as tc:
        _kernel_body(tc, xT, ctxT, wqT, wkT, wvT, woT, out)
    nc.compile()
    return nc


def _kernel_body(tc, xT, ctxT, wqT, wkT, wvT, woT, out):
    nc = tc.nc
    EXP = mybir.ActivationFunctionType.Exp
    NT = N // 512       # q-row tiles of 512
    MT = M // 128       # context-row tiles of 128
    CT = DIM // 128     # contraction tiles for projections
    DT = HG // 128      # head-dim tiles per core (= head pairs)

    with ExitStack() as ctx:
        sb = ctx.enter_context(tc.tile_pool(name="sb", bufs=1))

        xT_sb = sb.tile([128, CT, N], BF16, tag="xT")
        ctxT_sb = sb.tile([128, CT, M], BF16, tag="ctxT")
        wq_sb = sb.tile([128, CT, HG], BF16, tag="wq")
        wk_sb = sb.tile([128, CT, HG], BF16, tag="wk")
        wv_sb = sb.tile([128, CT, HG], BF16, tag="wv")
        wo_sb = sb.tile([128, DT, DIM], BF16, tag="wo")
        qT_sb = sb.tile([128, DT, N], BF16, tag="qT")
        kT_sb = sb.tile([128, DT, M], BF16, tag="kT")
        v_sb = sb.tile([128, MT, 8 * 65], BF16, tag="v")
        on_sb = sb.tile([128, DT, N], BF16, tag="on")

        # ---- loads (host passes bf16) ----
        for c in range(CT):
            nc.sync.dma_start(out=wq_sb[:, c, :], in_=wqT[c * 128:(c + 1) * 128, :])
        for c in range(CT):
            nc.sync.dma_start(out=xT_sb[:, c, :], in_=xT[c * 128:(c + 1) * 128, :])
        for c in range(CT):
            nc.sync.dma_start(out=wk_sb[:, c, :], in_=wkT[c * 128:(c + 1) * 128, :])
        for c in range(CT):
            nc.sync.dma_start(out=ctxT_sb[:, c, :], in_=ctxT[c * 128:(c + 1) * 128, :])
        for c in range(CT):
            nc.sync.dma_start(out=wv_sb[:, c, :], in_=wvT[c * 128:(c + 1) * 128, :])
        for t in range(DT):
            nc.sync.dma_start(out=wo_sb[:, t, :], in_=woT[t * 128:(t + 1) * 128, :])

        v_r = v_sb.rearrange("p m (h x) -> p m h x", x=65)
        for h in range(8):
            nc.vector.memset(v_r[:, :, h, 64:65], 1.0)

        # ---- compute: projections interleaved into attention ----
        # Pair 0's Q/K/V projections run up front; while attention for pair
        # pr streams (ScalarE-bound), the PE's idle slots are filled with
        # pair pr+1's projections, and during the last pair with the output
        # projection of already-normalized row blocks.
        psp = ctx.enter_context(tc.tile_pool(name="psp", bufs=2, space="PSUM"))
        pss = ctx.enter_context(tc.tile_pool(name="pss", bufs=2, space="PSUM"))
        pso = ctx.enter_context(tc.tile_pool(name="pso", bufs=2, space="PSUM"))
        sba = ctx.enter_context(tc.tile_pool(name="sba", bufs=6))
        sbn = ctx.enter_context(tc.tile_pool(name="sbn", bufs=4))
        sbo = ctx.enter_context(tc.tile_pool(name="sbo", bufs=3))

        def q_group(pr, jn):
            ps = psp.tile([128, 512], F32, tag="proj", name="qg")
            for c in range(CT):
                nc.tensor.matmul(
                    ps,
                    wq_sb[:, c, pr * 128:(pr + 1) * 128],
                    xT_sb[:, c, jn * 512:(jn + 1) * 512],
                    start=(c == 0), stop=(c == CT - 1),
                )
                if c % 3 == 2:
                    yield
            nc.vector.tensor_copy(qT_sb[:, pr, jn * 512:(jn + 1) * 512], ps)

        def k_group(pr, jm):
            ps = psp.tile([128, 512], F32, tag="proj", name="kg")
            for c in range(CT):
                nc.tensor.matmul(
                    ps,
                    wk_sb[:, c, pr * 128:(pr + 1) * 128],
                    ctxT_sb[:, c, jm * 512:(jm + 1) * 512],
                    start=(c == 0), stop=(c == CT - 1),
                )
                if c % 3 == 2:
                    yield
            nc.vector.tensor_copy(kT_sb[:, pr, jm * 512:(jm + 1) * 512], ps)

        def v_group(cp, i):
            # one couple = two head pairs = 4 heads (256 projection dims)
            ps = psp.tile([128, 256], F32, tag="proj", name="vg")
            for c in range(CT):
                nc.tensor.matmul(
                    ps,
                    ctxT_sb[:, c, i * 128:(i + 1) * 128],
                    wv_sb[:, c, cp * 256:(cp + 1) * 256],
                    start=(c == 0), stop=(c == CT - 1),
                )
                if c % 3 == 2:
                    yield
            nc.vector.tensor_copy(
                v_r[:, i, 4 * cp:4 * cp + 4, 0:64],
                ps.rearrange("p (h d) -> p h d", h=4),
            )

        def final_group(n128, e):
            ps = psp.tile([128, 512], F32, tag="proj", name="fg")
            for t in range(DT):
                nc.tensor.matmul(
                    ps,
                    on_sb[:, t, n128 * 128:(n128 + 1) * 128],
                    wo_sb[:, t, e * 512:(e + 1) * 512],
                    start=(t == 0), stop=(t == DT - 1),
                )
                if t == 1:
                    yield
            of = sbo.tile([128, 512], F32, tag="of", name="of")
            nc.vector.tensor_copy(of, ps)
            nc.sync.dma_start(
                out=out[n128 * 128:(n128 + 1) * 128, e * 512:(e + 1) * 512],
                in_=of,
            )

        def proj_pair_gens(pr, with_v):
            gens = ([q_group(pr, jn) for jn in range(NT)]
                    + [k_group(pr, jm) for jm in range(M // 512)])
            if with_v:
                # V for the NEXT couple (pairs 2,3), queued while pair 0 runs
                gens += [v_group(1, i) for i in range(MT)]
            return gens

        class Pacer:
            def __init__(self):
                self.queue = []
                self.cur = None

            def step_group(self):
                # emit one whole projection group (all its chunks)
                if self.cur is not None:
                    for _ in self.cur:
                        pass
                    self.cur = None
                elif self.queue:
                    for _ in self.queue.pop(0):
                        pass

            def step(self, n=1):
                for _ in range(n):
                    while True:
                        if self.cur is None:
                            if not self.queue:
                                return
                            self.cur = self.queue.pop(0)
                        try:
                            next(self.cur)
                            break
                        except StopIteration:
                            self.cur = None

            def drain(self):
                while self.cur is not None or self.queue:
                    self.step()

        def scores(pr, j, i):
            s = pss.tile([128, 1024], F32, tag="sc", name="sc")
            for half in range(2):
                lo, hi = half * 64, half * 64 + 64
                nc.tensor.matmul(
                    s[:, half * 512:(half + 1) * 512],
                    kT_sb[lo:hi, pr, i * 128:(i + 1) * 128],
                    qT_sb[lo:hi, pr, j * 512:(j + 1) * 512],
                    start=True, stop=True,
                )
            return s

        def exp_av(oo, pr, i, s):
            a = sba.tile([128, 1024], BF16, tag="attn", name="attn")
            nc.scalar.activation(a, s, EXP, scale=SCALE)
            for half in range(2):
                nc.tensor.matmul(
                    oo[half],
                    v_r[:, i, 2 * pr + half, :],
                    a[:, half * 512:(half + 1) * 512],
                    start=(i == 0), stop=(i == MT - 1),
                )

        def normalize(oo, pr, j):
            # rows 0..63 of oacc are sum(attn*v); row 64 is sum(attn).
            # Whole chain runs on DVE + GpSimd, off the PE/ACT critical path;
            # both reciprocals issue before the muls so the GpSimd broadcasts
            # overlap the second half's DVE work.
            bcs = []
            for half in range(2):
                den = sbn.tile([1, 512], F32, tag="den", name="den")
                nc.vector.tensor_copy(den, oo[half][64:65, :])
                rec32 = sbn.tile([1, 512], F32, tag="rec32", name="rec32")
                nc.vector.reciprocal_approx_fast(out=rec32, in_=den)
                bc = sbn.tile([64, 512], F32, tag="bc", name="bc")
                nc.gpsimd.partition_broadcast(bc, rec32)
                bcs.append(bc)
            for half in range(2):
                nc.vector.tensor_mul(
                    on_sb[half * 64:half * 64 + 64, pr, j * 512:(j + 1) * 512],
                    oo[half][0:64, :], bcs[half],
                )

        # pair 0 projections + V for couple 0 (heads 0-3) up front
        for g in proj_pair_gens(0, False) + [v_group(0, i) for i in range(MT)]:
            for _ in g:
                pass

        pacer = Pacer()
        for pr in range(DT):
            if pr + 1 < DT:
                pacer.queue.extend(proj_pair_gens(pr + 1, with_v=(pr == 0)))
            for j in range(NT):
                if pr == DT - 1 and j >= 1:
                    jj = j - 1  # tile (pr, jj) was normalized at its end
                    pacer.queue.extend(
                        final_group(n128, e)
                        for n128 in range(jj * 4, jj * 4 + 4)
                        for e in range(2))
                oo = [pso.tile([65, 512], F32, tag="oacc", name=f"oacc{h}")
                      for h in range(2)]
                fifo = [scores(pr, j, 0), scores(pr, j, 1)]
                for i in range(MT):
                    if i + 2 < MT:
                        fifo.append(scores(pr, j, i + 2))
                    exp_av(oo, pr, i, fifo.pop(0))
                    if i % 2 == 1:
                        pacer.step_group()
                normalize(oo, pr, j)
            pacer.drain()
        # last row block's output projection
        for n128 in range(12, 16):
            for e in range(2):
                for _ in final_group(n128, e):
                    pass


def kernel(x, context, Wq, Wk, Wv, Wo, bo):
    x = np.asarray(x, dtype=np.float32)
    context = np.asarray(context, dtype=np.float32)
    Wq = np.asarray(Wq, dtype=np.float32)
    Wk = np.asarray(Wk, dtype=np.float32)
    Wv = np.asarray(Wv, dtype=np.float32)
    Wo = np.asarray(Wo, dtype=np.float32)
    bo = np.asarray(bo, dtype=np.float32)

    if "nc" not in _CACHE:
        _CACHE["nc"] = _build_program()
    nc = _CACHE["nc"]

    in_maps = _make_in_maps(x, context, Wq, Wk, Wv, Wo)
    res = bass_utils.run_bass_kernel_spmd(nc, in_maps, core_ids=list(range(NCORES)))

    final = np.empty((B, N, DIM), dtype=np.float32)
    for b in range(B):
        final[b] = res.results[2 * b]["out"] + res.results[2 * b + 1]["out"] + bo
    return final


def _make_in_maps(x, context, Wq, Wk, Wv, Wo):
    bf = ml_dtypes.bfloat16
    xT = [np.ascontiguousarray(x[b].T).astype(bf) for b in range(B)]
    ctxT = [np.ascontiguousarray(context[b].T).astype(bf) for b in range(B)]
    wT = {}
    for g in range(2):
        sl = slice(g * HG, (g + 1) * HG)
        wT[g] = {
            "wqT": np.ascontiguousarray(Wq[sl, :].T).astype(bf),
            "wkT": np.ascontiguousarray(Wk[sl, :].T).astype(bf),
            "wvT": np.ascontiguousarray(Wv[sl, :].T).astype(bf),
            "woT": np.ascontiguousarray(Wo[:, sl].T).astype(bf),
        }
    in_maps = []
    for c in range(NCORES):
        b, g = c // 2, c % 2
        m = {"xT": xT[b], "ctxT": ctxT[b]}
        m.update(wT[g])
        in_maps.append(m)
    return in_maps


def timed_run(inp, trace_dir=None):
    """Run with NTFF tracing; returns HW exec time in ns (or None)."""
    if "nc" not in _CACHE:
        _CACHE["nc"] = _build_program()
    nc = _CACHE["nc"]
    in_maps = _make_in_maps(
        np.asarray(inp["x"], np.float32), np.asarray(inp["context"], np.float32),
        np.asarray(inp["Wq"], np.float32), np.asarray(inp["Wk"], np.float32),
        np.asarray(inp["Wv"], np.float32), np.asarray(inp["Wo"], np.float32))
    res = bass_utils.run_bass_kernel_spmd(
        nc, in_maps, core_ids=list(range(NCORES)), trace=True, tmpdir=trace_dir)
    return res.exec_time_ns


# revision 33
# speedup vs baseline: 1.2056x; 1.0231x over previous
"""Cross-attention Trainium2 kernel (self-contained).

Reference computation (B=4, N=M=2048, DIM=1024, H=16, Dh=64):
    q = x @ Wq.T ; k = ctx @ Wk.T ; v = ctx @ Wv.T       (per-head split)
    out = softmax(q k^T / sqrt(Dh)) v                     (per b, h)
    final = out @ Wo.T + bo

Sharding over 8 NeuronCores: core c -> (batch b = c//2, head-group g = c%2).
Each core handles 8 heads (512 of the 1024 inner dims) of one batch and
produces a partial (2048, 1024) output-projection contribution; the host sums
the two partials per batch and adds the bias.

On-chip dataflow keeps every matmul contraction on the partition axis:
    Q^T = (Wq_g^T as lhsT stacks) with x^T as moving operand -> (d, n)
    K^T likewise -> (d, m);  V -> (m, d) with a ones-column per head so the
    attn@V matmul also emits softmax denominators.
    scores^T (m, n) per head via K=64 matmuls, two heads packed in the
    128-row PE array; exp on ScalarE with fused 1/sqrt(Dh) scale (max |logit|
    = 3.8, so no max-subtraction needed); denominator reciprocal broadcast via
    a rank-1 PE matmul.
"""

import numpy as np
import ml_dtypes
from contextlib import ExitStack

import concourse.bass as bass
import concourse.bacc as bacc
import concourse.tile as tile
from concourse import mybir
from concourse import bass_utils

F32 = mybir.dt.float32
BF16 = mybir.dt.bfloat16

B, N, M, DIM = 4, 2048, 2048, 1024
H, DH = 16, 64
NCORES = 8
HG = DIM // 2          # head dims per core (8 heads * 64)
SCALE = DH ** -0.5

_CACHE = {}


def _build_program():
    nc = bacc.Bacc(
        "TRN2",
        target_bir_lowering=False,
        debug=False,
        enable_asserts=False,
        num_devices=NCORES,
    )
    xT = nc.dram_tensor("xT", (DIM, N), BF16, kind="ExternalInput").ap()
    ctxT = nc.dram_tensor("ctxT", (DIM, M), BF16, kind="ExternalInput").ap()
    wqT = nc.dram_tensor("wqT", (DIM, HG), BF16, kind="ExternalInput").ap()
    wkT = nc.dram_tensor("wkT", (DIM, HG), BF16, kind="ExternalInput").ap()
    wvT = nc.dram_tensor("wvT", (DIM, HG), BF16, kind="ExternalInput").ap()
    woT = nc.dram_tensor("woT", (HG, DIM), BF16, kind="ExternalInput").ap()
    out = nc.dram_tensor("out", (N, DIM), F32, kind="ExternalOutput").ap()

    with tile.TileContext(nc) as tc:
        _kernel_body(tc, xT, ctxT, wqT, wkT, wvT, woT, out)
    nc.compile()
    return nc


def _kernel_body(tc, xT, ctxT, wqT, wkT, wvT, woT, out):
    nc = tc.nc
    EXP = mybir.ActivationFunctionType.Exp
    NT = N // 512       # q-row tiles of 512
    MT = M // 128       # context-row tiles of 128
    CT = DIM // 128     # contraction tiles for projections
    DT = HG // 128      # head-dim tiles per core (= head pairs)

    with ExitStack() as ctx:
        sb = ctx.enter_context(tc.tile_pool(name="sb", bufs=1))

        xT_sb = sb.tile([128, CT, N], BF16, tag="xT")
        ctxT_sb = sb.tile([128, CT, M], BF16, tag="ctxT")
        wq_sb = sb.tile([128, CT, HG], BF16, tag="wq")
        wk_sb = sb.tile([128, CT, HG], BF16, tag="wk")
        wv_sb = sb.tile([128, CT, HG], BF16, tag="wv")
        wo_sb = sb.tile([128, DT, DIM], BF16, tag="wo")
        qT_sb = sb.tile([128, DT, N], BF16, tag="qT")
        kT_sb = sb.tile([128, DT, M], BF16, tag="kT")
        v_sb = sb.tile([128, MT, 8 * 65], BF16, tag="v")
        on_sb = sb.tile([128, DT, N], BF16, tag="on")

        # ---- loads (host passes bf16) ----
        for c in range(CT):
            nc.sync.dma_start(out=wq_sb[:, c, :], in_=wqT[c * 128:(c + 1) * 128, :])
        for c in range(CT):
            nc.sync.dma_start(out=xT_sb[:, c, :], in_=xT[c * 128:(c + 1) * 128, :])
        for c in range(CT):
            nc.sync.dma_start(out=wk_sb[:, c, :], in_=wkT[c * 128:(c + 1) * 128, :])
        for c in range(CT):
            nc.sync.dma_start(out=ctxT_sb[:, c, :], in_=ctxT[c * 128:(c + 1) * 128, :])
        for c in range(CT):
            nc.sync.dma_start(out=wv_sb[:, c, :], in_=wvT[c * 128:(c + 1) * 128, :])
        for t in range(DT):
            nc.sync.dma_start(out=wo_sb[:, t, :], in_=woT[t * 128:(t + 1) * 128, :])

        v_r = v_sb.rearrange("p m (h x) -> p m h x", x=65)
        for h in range(8):
            nc.vector.memset(v_r[:, :, h, 64:65], 1.0)

        # ---- compute: projections interleaved into attention ----
        # Pair 0's Q/K/V projections run up front; while attention for pair
        # pr streams (ScalarE-bound), the PE's idle slots are filled with
        # pair pr+1's projections, and during the last pair with the output
        # projection of already-normalized row blocks.
        psp = ctx.enter_context(tc.tile_pool(name="psp", bufs=2, space="PSUM"))
        pss = ctx.enter_context(tc.tile_pool(name="pss", bufs=2, space="PSUM"))
        pso = ctx.enter_context(tc.tile_pool(name="pso", bufs=2, space="PSUM"))
        sba = ctx.enter_context(tc.tile_pool(name="sba", bufs=6))
        sbn = ctx.enter_context(tc.tile_pool(name="sbn", bufs=4))
        sbo = ctx.enter_context(tc.tile_pool(name="sbo", bufs=3))

        def q_group(pr, jn):
            ps = psp.tile([128, 512], F32, tag="proj", name="qg")
            for c in range(CT):
                nc.tensor.matmul(
                    ps,
                    wq_sb[:, c, pr * 128:(pr + 1) * 128],
                    xT_sb[:, c, jn * 512:(jn + 1) * 512],
                    start=(c == 0), stop=(c == CT - 1),
                )
                if c % 3 == 2:
                    yield
            nc.vector.tensor_copy(qT_sb[:, pr, jn * 512:(jn + 1) * 512], ps)

        def k_group(pr, jm):
            ps = psp.tile([128, 512], F32, tag="proj", name="kg")
            for c in range(CT):
                nc.tensor.matmul(
                    ps,
                    wk_sb[:, c, pr * 128:(pr + 1) * 128],
                    ctxT_sb[:, c, jm * 512:(jm + 1) * 512],
                    start=(c == 0), stop=(c == CT - 1),
                )
                if c % 3 == 2:
                    yield
            nc.vector.tensor_copy(kT_sb[:, pr, jm * 512:(jm + 1) * 512], ps)

        def v_group(cp, i):
            # one couple = two head pairs = 4 heads (256 projection dims)
            ps = psp.tile([128, 256], F32, tag="proj", name="vg")
            for c in range(CT):
                nc.tensor.matmul(
                    ps,
                    ctxT_sb[:, c, i * 128:(i + 1) * 128],
                    wv_sb[:, c, cp * 256:(cp + 1) * 256],
                    start=(c == 0), stop=(c == CT - 1),
                )
                if c % 3 == 2:
                    yield
            nc.vector.tensor_copy(
                v_r[:, i, 4 * cp:4 * cp + 4, 0:64],
                ps.rearrange("p (h d) -> p h d", h=4),
            )

        def final_group(n128, e):
            ps = psp.tile([128, 512], F32, tag="proj", name="fg")
            for t in range(DT):
                nc.tensor.matmul(
                    ps,
                    on_sb[:, t, n128 * 128:(n128 + 1) * 128],
                    wo_sb[:, t, e * 512:(e + 1) * 512],
                    start=(t == 0), stop=(t == DT - 1),
                )
                if t == 1:
                    yield
            of = sbo.tile([128, 512], F32, tag="of", name="of")
            nc.vector.tensor_copy(of, ps)
            nc.sync.dma_start(
                out=out[n128 * 128:(n128 + 1) * 128, e * 512:(e + 1) * 512],
                in_=of,
            )

        def proj_pair_gens(pr, with_v):
            gens = ([q_group(pr, jn) for jn in range(NT)]
                    + [k_group(pr, jm) for jm in range(M // 512)])
            if with_v:
                # V for the NEXT couple (pairs 2,3), queued while pair 0 runs
                gens += [v_group(1, i) for i in range(MT)]
            return gens

        class Pacer:
            def __init__(self):
                self.queue = []
                self.cur = None

            def step_group(self):
                # emit one whole projection group (all its chunks)
                if self.cur is not None:
                    for _ in self.cur:
                        pass
                    self.cur = None
                elif self.queue:
                    for _ in self.queue.pop(0):
                        pass

            def step(self, n=1):
                for _ in range(n):
                    while True:
                        if self.cur is None:
                            if not self.queue:
                                return
                            self.cur = self.queue.pop(0)
                        try:
                            next(self.cur)
                            break
                        except StopIteration:
                            self.cur = None

            def drain(self):
                while self.cur is not None or self.queue:
                    self.step()

        def scores(pr, j, i):
            s = pss.tile([128, 1024], F32, tag="sc", name="sc")
            for half in range(2):
                lo, hi = half * 64, half * 64 + 64
                nc.tensor.matmul(
                    s[:, half * 512:(half + 1) * 512],
                    kT_sb[lo:hi, pr, i * 128:(i + 1) * 128],
                    qT_sb[lo:hi, pr, j * 512:(j + 1) * 512],
                    start=True, stop=True,
                )
            return s

        def exp_av(oo, pr, i, s):
            a = sba.tile([128, 1024], BF16, tag="attn", name="attn")
            nc.scalar.activation(a, s, EXP, scale=SCALE)
            for half in range(2):
                nc.tensor.matmul(
                    oo[half],
                    v_r[:, i, 2 * pr + half, :],
                    a[:, half * 512:(half + 1) * 512],
                    start=(i == 0), stop=(i == MT - 1),
                )

        def normalize(oo, pr, j):
            # rows 0..63 of oacc are sum(attn*v); row 64 is sum(attn).
            # Whole chain runs on DVE + GpSimd, off the PE/ACT critical path;
            # both reciprocals issue before the muls so the GpSimd broadcasts
            # overlap the second half's DVE work.
            bcs = []
            for half in range(2):
                den = sbn.tile([1, 512], F32, tag="den", name="den")
                nc.vector.tensor_copy(den, oo[half][64:65, :])
                rec32 = sbn.tile([1, 512], F32, tag="rec32", name="rec32")
                nc.vector.reciprocal_approx_fast(out=rec32, in_=den)
                bc = sbn.tile([64, 512], F32, tag="bc", name="bc")
                nc.gpsimd.partition_broadcast(bc, rec32)
                bcs.append(bc)
            for half in range(2):
                nc.vector.tensor_mul(
                    on_sb[half * 64:half * 64 + 64, pr, j * 512:(j + 1) * 512],
                    oo[half][0:64, :], bcs[half],
                )

        # pair 0 Q/K and the first V tiles up front; the rest of couple 0's
        # V projection is paced into the first attention tile (safe: v[i] is
        # consumed at step i, and pacing stays ahead of it)
        for g in proj_pair_gens(0, False) + [v_group(0, i) for i in range(6)]:
            for _ in g:
                pass

        pacer = Pacer()
        pacer.queue.extend(v_group(0, i) for i in range(6, MT))
        for pr in range(DT):
            if pr + 1 < DT:
                pacer.queue.extend(proj_pair_gens(pr + 1, with_v=(pr == 0)))
            for j in range(NT):
                if pr == DT - 1 and j >= 1:
                    jj = j - 1  # tile (pr, jj) was normalized at its end
                    pacer.queue.extend(
                        final_group(n128, e)
                        for n128 in range(jj * 4, jj * 4 + 4)
                        for e in range(2))
                oo = [pso.tile([65, 512], F32, tag="oacc", name=f"oacc{h}")
                      for h in range(2)]
                fifo = [scores(pr, j, 0), scores(pr, j, 1)]
                for i in range(MT):
                    if i + 2 < MT:
                        fifo.append(scores(pr, j, i + 2))
                    exp_av(oo, pr, i, fifo.pop(0))
                    if (i % 2 == 1 and i < 15) or (pr == 0 and j == 0):
                        pacer.step_group()
                normalize(oo, pr, j)
            pacer.drain()
        # last row block's output projection
        for n128 in range(12, 16):
            for e in range(2):
                for _ in final_group(n128, e):
                    pass


def kernel(x, context, Wq, Wk, Wv, Wo, bo):
    x = np.asarray(x, dtype=np.float32)
    context = np.asarray(context, dtype=np.float32)
    Wq = np.asarray(Wq, dtype=np.float32)
    Wk = np.asarray(Wk, dtype=np.float32)
    Wv = np.asarray(Wv, dtype=np.float32)
    Wo = np.asarray(Wo, dtype=np.float32)
    bo = np.asarray(bo, dtype=np.float32)

    if "nc" not in _CACHE:
        _CACHE["nc"] = _build_program()
    nc = _CACHE["nc"]

    in_maps = _make_in_maps(x, context, Wq, Wk, Wv, Wo)
    res = bass_utils.run_bass_kernel_spmd(nc, in_maps, core_ids=list(range(NCORES)))

    final = np.empty((B, N, DIM), dtype=np.float32)
    for b in range(B):
        final[b] = res.results[2 * b]["out"] + res.results[2 * b + 1]["out"] + bo
    return final


def _make_in_maps(x, context, Wq, Wk, Wv, Wo):
    bf = ml_dtypes.bfloat16
    xT = [np.ascontiguousarray(x[b].T).astype(bf) for b in range(B)]
    ctxT = [np.ascontiguousarray(context[b].T).astype(bf) for b in range(B)]
    wT = {}
    for g in range(2):
        sl = slice(g * HG, (g + 1) * HG)
        wT[g] = {
            "wqT": np.ascontiguousarray(Wq[sl, :].T).astype(bf),
            "wkT": np.ascontiguousarray(Wk[sl, :].T).astype(bf),
            "wvT": np.ascontiguousarray(Wv[sl, :].T).astype(bf),
            "woT": np.ascontiguousarray(Wo[:, sl].T).astype(bf),
        }
    in_maps = []
    for c in range(NCORES):
        b, g = c // 2, c % 2
        m = {"xT": xT[b], "ctxT": ctxT[b]}
        m.update(wT[g])
        in_maps.append(m)
    return in_maps


def timed_run(inp, trace_dir=None):
    """Run with NTFF tracing; returns HW exec time in ns (or None)."""
    if "nc" not in _CACHE:
        _CACHE["nc"] = _build_program()
    nc = _CACHE["nc"]
    in_maps = _make_in_maps(
        np.asarray(inp["x"], np.float32), np.asarray(inp["context"], np.float32),
        np.asarray(inp["Wq"], np.float32), np.asarray(inp["Wk"], np.float32),
        np.asarray(inp["Wv"], np.float32), np.asarray(inp["Wo"], np.float32))
    res = bass_utils.run_bass_kernel_spmd(
        nc, in_maps, core_ids=list(range(NCORES)), trace=True, tmpdir=trace_dir)
    return res.exec_time_ns
